# revision 41
# baseline (speedup 1.0000x reference)
"""BroadcastAttention Trainium2 kernel (8 NeuronCores, data-parallel over batch).

Math per sample (C=512, N=4096, H=8 heads, HD=64):
    qkv = Wqkv @ x            # [H*(1+2HD), N]
    q[h,n], k[h,d,n], v[h,d,n] split per head
    s = softmax(q over n)     # [H, N]
    ctx[h,d] = sum_n k[h,d,n]*s[h,n]
    out = Wp @ (relu(v)*ctx) + bp

Key algebraic restructuring vs the straightforward formulation: the dense
K projection (a full [512,512]@[512,4096] matmul per sample, one third of
the FLOPs) is never computed. Since ctx_h = Wk_h @ (x @ s_h), we compute
t[c,h] = sum_n x[c,n]*exp(q[h,n]) and apply Wk to the tiny [C,H] result
(eselB broadcast matmul + fused multiply-reduce against Wk), with 1/Z
applied via a small PE/DVE chain. ctx is folded into the P-phase weights
(wps = wp * ctx per contraction channel) so the V phase never waits on it.

Scheduling (derived from perfetto traces; ~175us HW, PE busy ~133us):
    - Matmul operands are bf16 (fp32 streams the PE at half rate); PSUM
      stays fp32. Steady 512-col matmuls run at the ~216ns streaming
      floor. x is converted to bf16 ON THE HOST (identical precision to
      the old on-device cast, half the HBM bytes, no stage/cast
      pipeline); y is stored bf16 and upcast on the host. Total rel err
      ~5e-3 vs the 2e-2 gate.
    - x loads DMA straight into x_sb slices on the Sync HWDGE queue in
      strict need-order (heads column-major for compute start, tails
      row-major so whole rows finish early). One queue only: concurrent
      loads on two queues split HBM bandwidth and delay first-needed
      tiles; GpSimd SWDGE transfers run at ~half the HWDGE rate (fine
      for weights, not for x).
    - x -> xT via XBAR dma_start_transpose, FULL-ROW only (each call
      has a ~5us fixed cost regardless of size), issued ONLY from the
      Sync queue: issuing from the Activation HWDGE queue produces
      corrupt data (measured ~50% error on affected samples), and
      fine-grained sliced transposes raced their readers.
    - The 8 t-accumulation groups (M=8 matmuls at distinct 32-col
      tile_position groups into one pre-zeroed PSUM bank) must run
      CONTIGUOUSLY on the PE: interleaving other matmul chains between
      the open accumulation subgroups corrupts ctx.
    - The Tile scheduler is dataflow-driven and hoists ready work into
      PE gaps: the next sample's loads are issued column-major (first
      column-block on GpSimd) so its q pass becomes runnable while the
      current sample's ctx chain drains; its transposes are drained
      into the P-phase emission. V7 out-tiles are emitted around the
      z/ctx chain's tiny PE ops, V7 evictions all go to Scalar so
      Vector runs the chain unobstructed, and the wps fold is pipelined
      per-column across Scalar/Vector (it is the last gate before P).
    - dma_start issue costs ~0.6us of sequencer time and the DMA ring
      paces issues at the transfer rate, so everything queued behind a
      blocked transpose ucode is delayed - queue assignment and
      emission order are the main scheduling tools. y stores issue from
      GpSimd so o_sb recycling never waits on transpose ucode.
    - After a device wedge (NRT errors), the next run can silently
      return wrong results on some cores - rerun before trusting a
      rel-err measurement.
"""

import sys

for _p in ("/opt/trn_rl_repo",):
    if _p not in sys.path:
        sys.path.insert(0, _p)

from contextlib import ExitStack

import ml_dtypes
import numpy as np

import concourse.bass as bass
import concourse.mybir as mybir
import concourse.tile as tile
from concourse import bacc
from concourse.bass_utils import run_bass_kernel_spmd

# Problem constants (hardcoded per contract; kernel.py must be self-contained).
B, C, N = 16, 512, 4096
H, HD = 8, 64
NCORES = 8
BPC = B // NCORES  # samples per core
CT = C // 128      # 4 contraction/partition tiles of 128
NT = N // 128      # 32 n-tiles
FREE = 512         # matmul moving free-dim chunk
NCH = N // FREE    # 8 chunks
FP = mybir.dt.float32
BF = mybir.dt.bfloat16

# Results of the last run (for test harness introspection).
LAST_RESULTS = None


def _build(has_qkv_bias: bool, has_p_bias: bool) -> bass.Bass:
    nc = bacc.Bacc("TRN2", target_bir_lowering=False, debug=False)

    x_d = nc.declare_dram_parameter("x", [BPC, C, N], BF, isOutput=False)
    wq_d = nc.declare_dram_parameter("wqT", [C, H], BF, isOutput=False)
    wv_d = nc.declare_dram_parameter("wvT", [C, C], BF, isOutput=False)
    wk_d = nc.declare_dram_parameter("wkO", [C, C], BF, isOutput=False)
    wp_d = nc.declare_dram_parameter("wpT", [C, C], BF, isOutput=False)
    eselB_d = nc.declare_dram_parameter("eselB", [128, C], BF, isOutput=False)
    eselT_d = nc.declare_dram_parameter("eselT", [H, 128], FP, isOutput=False)
    bq_d = nc.declare_dram_parameter("bq", [1, H], BF, isOutput=False)
    bkc_d = nc.declare_dram_parameter("bkcol", [C], FP, isOutput=False)
    bv_d = nc.declare_dram_parameter("bv", [1, C], BF, isOutput=False)
    bp_d = nc.declare_dram_parameter("bp", [C], FP, isOutput=False)
    y_d = nc.declare_dram_parameter("y", [BPC, C, N], BF, isOutput=True)

    AF = mybir.ActivationFunctionType
    OP = mybir.AluOpType

    with tile.TileContext(nc) as tc, ExitStack() as ctx:
        consts = ctx.enter_context(tc.tile_pool(name="consts", bufs=1))
        xpool = ctx.enter_context(tc.tile_pool(name="xpool", bufs=2))
        xtpool = ctx.enter_context(tc.tile_pool(name="xtpool", bufs=1))
        apool = ctx.enter_context(tc.tile_pool(name="apool", bufs=1))
        spool = ctx.enter_context(tc.tile_pool(name="spool", bufs=2))
        wpspool = ctx.enter_context(tc.tile_pool(name="wpspool", bufs=2))
        opool = ctx.enter_context(tc.tile_pool(name="opool", bufs=10))
        small = ctx.enter_context(tc.tile_pool(name="small", bufs=2))
        # 5 rotating matmul banks (the eselB finalize matmuls share the
        # same pool): deeper PSUM rotation hides the V/P chain-head
        # stationary loads + bank-acquire waits.
        ps_q = ctx.enter_context(tc.tile_pool(name="ps_q", bufs=2, space="PSUM"))
        ps_ctx = ctx.enter_context(tc.tile_pool(name="ps_ctx", bufs=1, space="PSUM"))
        ps_mm = ctx.enter_context(tc.tile_pool(name="ps_mm", bufs=5, space="PSUM"))

        # ---- constants / weights into SBUF ----
        # GpSimd (SWDGE) queue: startup weights, away from x on Sync.
        wq_sb = consts.tile([128, CT, H], BF)
        wv_sb = consts.tile([128, CT, C], BF)
        wk_sb = consts.tile([128, CT, C], BF)
        wp_sb = consts.tile([128, CT, C], BF)
        eselB_sb = consts.tile([128, C], BF)
        eselT_sb = consts.tile([H, 128], FP)
        ones_col = consts.tile([128, 1], FP)

        # All weights load upfront on GpSimd (SWDGE) — the slower SWDGE
        # transfer rate (~half of HWDGE) is fine for weights, and this
        # keeps the Sync HWDGE queue free for x loads + transposes.
        nc.gpsimd.memset(ones_col[:], 1.0)
        for wct in range(CT):
            wsl = slice(wct * 128, (wct + 1) * 128)
            nc.gpsimd.dma_start(out=wq_sb[:, wct, :], in_=wq_d[wsl, :])
            nc.gpsimd.dma_start(out=wv_sb[:, wct, :], in_=wv_d[wsl, :])
        nc.gpsimd.dma_start(out=eselB_sb[:], in_=eselB_d[:, :])
        nc.gpsimd.dma_start(out=eselT_sb[:], in_=eselT_d[:, :])
        for wct in range(CT):
            wsl = slice(wct * 128, (wct + 1) * 128)
            nc.gpsimd.dma_start(out=wk_sb[:, wct, :], in_=wk_d[wsl, :])
            nc.gpsimd.dma_start(out=wp_sb[:, wct, :], in_=wp_d[wsl, :])
        if has_qkv_bias:
            bq_sb = consts.tile([1, H], BF)
            bkc_sb = consts.tile([128, CT], FP)
            bv_sb = consts.tile([1, C], BF)
            ones_row = consts.tile([1, FREE], BF)
            nc.gpsimd.dma_start(out=bq_sb[:], in_=bq_d[:, :])
            nc.gpsimd.dma_start(
                out=bkc_sb[:], in_=bkc_d.rearrange("(o p) -> p o", p=128)
            )
            nc.gpsimd.dma_start(out=bv_sb[:], in_=bv_d[:, :])
            nc.gpsimd.memset(ones_row[:], 1.0)
        if has_p_bias:
            bp_sb = consts.tile([128, CT], FP)
            nc.gpsimd.dma_start(
                out=bp_sb[:], in_=bp_d.rearrange("(o p) -> p o", p=128)
            )

        # x arrives bf16 from the host (identical precision to the old
        # on-device fp32->bf16 cast, but half the HBM bytes and no
        # stage/cast pipeline at all). Loads go straight into x_sb
        # slices in [128, 1024] pieces, need-ordered. Each
        # dma_start_transpose has a ~5us fixed cost regardless of size,
        # so transposes stay FULL-ROW (4 per sample, Sync ONLY — the
        # Activation HWDGE queue corrupts transposes).
        QW = 1024  # quarter width

        def emit_ld(b, x_sb, r, p, queue):
            queue.dma_start(
                out=x_sb[:, r, p * QW:(p + 1) * QW],
                in_=x_d[b, r * 128:(r + 1) * 128, p * QW:(p + 1) * QW],
            )

        def emit_T(x_sb, xT_sb, r, queue=None):
            # XBAR transpose of one full x row-block (identical op to the
            # proven full-sample design, just issued earlier).
            (queue or nc.sync).dma_start_transpose(
                out=xT_sb[:, :, r * 128:(r + 1) * 128],
                in_=x_sb[:, r, :],
            )

        def emit_t_group(chk, scoresT, xT_sb, ctx_big):
            # 4 M=8 matmuls at distinct 32-col tile_position groups run
            # concurrently; accumulate into the pre-zeroed ctx_big bank.
            for j in range(4):
                nt = chk * 4 + j
                nc.tensor.matmul(
                    ctx_big[32 * j:32 * j + H, :],
                    scoresT[:, nt, :], xT_sb[:, nt, :],
                    start=False, stop=(chk == NCH - 1),
                    skip_group_check=True,
                    tile_position=(0, 32 * j),
                )

        def alloc_sample():
            xT_sb = xtpool.tile([128, NT, C], BF, tag="xT", name="xT_sb")
            scoresT = spool.tile([128, NT, H], BF, tag="scoresT", name="scoresT")
            a_sb = apool.tile([128, CT, N], BF, tag="a_sb", name="a_sb")
            wps_sb = wpspool.tile([128, CT, C], BF, tag="wps", name="wps_sb")
            ctx_big = ps_ctx.tile([128, C], FP, tag="ctx", name="ctx_big")
            nc.vector.memset(ctx_big[:], 0.0)
            return xT_sb, scoresT, a_sb, wps_sb, ctx_big

        x_sb = xpool.tile([128, CT, N], BF, tag="x_sb", name="x_sb")
        cur = alloc_sample()

        for b in range(BPC):
            xT_sb, scoresT, a_sb, wps_sb, ctx_big = cur

            if b == 0:
                # ALL of batch 0's x loads issue upfront on Sync (HWDGE
                # — SWDGE transfers at only ~half the rate) in strict
                # need-order: heads (cols 0-2048) column-major so the
                # chunk loop starts ASAP, then tails row-major so whole
                # rows complete early. The 4 full-row transposes follow
                # on Sync — each blocks the queue head until its row's
                # loads land, which is harmless (nothing else needs
                # Sync until the next sample's loads). Transposes MUST
                # issue from Sync: dma_start_transpose from the
                # Activation queue produces corrupt data (measured:
                # every sample whose odd rows transposed via Scalar
                # came out ~50% wrong; all-Sync samples were exact).
                for p in (0, 1):
                    for r in range(CT):
                        emit_ld(b, x_sb, r, p, nc.sync)
                for r in range(CT):
                    # one [128,2048] DMA per row tail: 12 total issues
                    # keeps the whole load burst inside the DMA ring's
                    # ~8-9 immediate-issue depth, so the Sync queue head
                    # reaches the transposes ~7us sooner.
                    nc.sync.dma_start(
                        out=x_sb[:, r, 2 * QW:4 * QW],
                        in_=x_d[b, r * 128:(r + 1) * 128, 2 * QW:4 * QW],
                    )
                for r in range(CT):
                    # Transpose straight from DRAM (x is bf16 in HBM):
                    # no dependency on the SBUF loads, so each ucode
                    # starts at queue-head-reach instead of waiting for
                    # its row to land; the extra HBM read overlaps the
                    # otherwise-idle window after batch 0's loads.
                    nc.sync.dma_start_transpose(
                        out=xT_sb[:, :, r * 128:(r + 1) * 128],
                        in_=x_d[b, r * 128:(r + 1) * 128, :],
                    )

            for chk in range(NCH):
                # ---- q pass for the 4 n-tiles of this chunk ----
                for j4 in range(4):
                    nt = chk * 4 + j4
                    nsl = slice(nt * 128, (nt + 1) * 128)
                    q_ps = ps_q.tile([128, H], FP, tag="q8", name="q_ps")
                    for ct in range(CT):
                        xsl = x_sb[:, ct, nsl]
                        last = (ct == CT - 1) and not has_qkv_bias
                        nc.tensor.matmul(
                            q_ps[:], xsl, wq_sb[:, ct, :],
                            start=(ct == 0), stop=last,
                        )
                    if has_qkv_bias:
                        nc.tensor.matmul(
                            q_ps[:], ones_row[:, 0:128], bq_sb[:],
                            start=False, stop=True,
                        )
                    nc.scalar.activation(
                        out=scoresT[:, nt, :], in_=q_ps[:], func=AF.Exp
                    )

                def emit_V(i_list, scalar_evict=False):
                    csl = slice(chk * FREE, (chk + 1) * FREE)
                    for i in i_list:
                        v_ps = ps_mm.tile(
                            [128, FREE], FP, tag="mm512", name="v_ps"
                        )
                        for ct in range(CT):
                            last = (ct == CT - 1) and not has_qkv_bias
                            nc.tensor.matmul(
                                v_ps[:],
                                wv_sb[:, ct, i * 128:(i + 1) * 128],
                                x_sb[:, ct, csl],
                                start=(ct == 0), stop=last,
                            )
                        if has_qkv_bias:
                            nc.tensor.matmul(
                                v_ps[:], bv_sb[:, i * 128:(i + 1) * 128],
                                ones_row[:], start=False, stop=True,
                            )
                        if i % 2 == 0 and not scalar_evict:
                            nc.vector.tensor_scalar_max(
                                out=a_sb[:, i, csl], in0=v_ps[:], scalar1=0.0
                            )
                        else:
                            nc.scalar.activation(
                                out=a_sb[:, i, csl], in_=v_ps[:], func=AF.Relu
                            )

                if chk < NCH - 1:
                    # ---- V phase for this chunk (no ctx dependency) ----
                    emit_V(range(CT))
                    continue

                # ---- last chunk: V out-tiles 0-2 run FIRST so the PE
                # chews them while the last transpose lands (t-groups
                # need all of xT); then the 8 t-groups contiguously
                # (other matmul chains interleaved between the open
                # tile_position accumulation groups corrupt ctx); then
                # the z/ctx chain with V(3) slotted between its tiny PE
                # ops to cover the DVE round trips. V7's evictions all
                # go to Scalar so Vector runs the chain unobstructed. ----
                zpart = small.tile([128, H], FP, tag="zpart", name="zpart")
                nc.vector.reduce_sum(
                    out=zpart[:],
                    in_=scoresT[:].rearrange("p nt h -> p h nt"),
                    axis=mybir.AxisListType.X,
                )
                emit_V([0, 1, 2], scalar_evict=True)
                for g in range(NCH):
                    emit_t_group(g, scoresT, xT_sb, ctx_big)
                z_ps = ps_q.tile([H, 1], FP, tag="q8", name="z_ps")
                nc.tensor.matmul(
                    z_ps[:], zpart[:], ones_col[:], start=True, stop=True
                )
                invz = small.tile([H, 1], FP, tag="invz", name="invz")
                nc.vector.reciprocal(out=invz[:], in_=z_ps[:])
                emit_V([3], scalar_evict=True)
                zrow_ps = ps_q.tile([128, 1], FP, tag="q8", name="zrow_ps")
                nc.tensor.matmul(
                    zrow_ps[:], eselT_sb[:], invz[:], start=True, stop=True
                )
                zrow = small.tile([128, 1], FP, tag="zrow", name="zrow")
                nc.vector.tensor_copy(out=zrow[:], in_=zrow_ps[:])
                ctxcopy = small.tile([128, C], BF, tag="ctxcopy", name="ctxcopy")
                nc.vector.tensor_scalar_mul(
                    out=ctxcopy[:], in0=ctx_big[:], scalar1=zrow[:]
                )
                # ctx[ch] = sum_c Wk[ch,c] * t[c,h(ch)]: broadcast t to
                # all channels of its head (eselB matmul, which also
                # combines the 4 substreams), then a fused elementwise
                # multiply-reduce against Wk.
                # ctxv held as 4 separate [128,1] tiles so the wps fold
                # can scale with full-tile scalar operands on either
                # engine; the fold is pipelined right behind each
                # reduce (it is the last gate before the P phase).
                for i in range(CT):
                    tb_ps = ps_mm.tile([128, C], FP, tag="mm512", name="tb_ps")
                    nc.tensor.matmul(
                        tb_ps[:], eselB_sb[:, i * 128:(i + 1) * 128],
                        ctxcopy[:], start=True, stop=True,
                    )
                    junk = small.tile([128, C], BF, tag="junk", name="junk")
                    nc.vector.tensor_tensor(
                        out=junk[:], in0=tb_ps[:], in1=wk_sb[:, i, :],
                        op=OP.mult,
                    )
                    ctxv_i = small.tile([128, 1], FP, tag="ctxv", name="ctxv")
                    nc.vector.reduce_sum(
                        out=ctxv_i[:], in_=junk[:],
                        axis=mybir.AxisListType.X,
                    )
                    if has_qkv_bias:
                        ctxvb_i = small.tile(
                            [128, 1], FP, tag="ctxv", name="ctxvb"
                        )
                        nc.vector.tensor_tensor(
                            out=ctxvb_i[:], in0=ctxv_i[:],
                            in1=bkc_sb[:, i:i + 1], op=OP.add,
                        )
                        ctxv_i = ctxvb_i
                    if i % 2 == 0:
                        nc.scalar.activation(
                            out=wps_sb[:, i, :], in_=wp_sb[:, i, :],
                            func=AF.Identity, scale=ctxv_i[:],
                        )
                    else:
                        nc.vector.tensor_scalar_mul(
                            out=wps_sb[:, i, :], in0=wp_sb[:, i, :],
                            scalar1=ctxv_i[:],
                        )

            # ---- prefetch next sample: HBM loads row-major on the idle
            # GpSimd SWDGE queue; casts + full-row transposes are
            # interleaved into the P-phase chunk slots below (32 slots
            # hold 16 casts + 4 transposes), so the next chunk loop
            # starts with x and xT fully resident. ----
            nxt = b + 1 if b + 1 < BPC else None
            interleave = []
            if nxt is not None:
                # Column-major, first column-block on GpSimd: the next
                # sample's q pass for early chunks becomes runnable
                # while this sample's ctx chain drains, and the
                # dataflow scheduler hoists it into those PE gaps.
                x_sb = xpool.tile([128, CT, N], BF, tag="x_sb", name="x_sb")
                cur = alloc_sample()
                nxT = cur[0]
                for p in range(4):
                    for r in range(CT):
                        emit_ld(nxt, x_sb, r, p,
                                nc.gpsimd if p == 0 else nc.sync)
                for r in range(CT):
                    interleave.append((emit_T, (x_sb, nxT, r)))
            ivi = [0]

            def drain_interleave(k=1):
                for _ in range(k):
                    if ivi[0] < len(interleave):
                        fn, args = interleave[ivi[0]]
                        fn(*args)
                        ivi[0] += 1

            # ---- P phase: output projection (ctx-folded weights) ----
            HSTG = N // 2
            for o in range(CT):
                for half in range(2):
                    o_sb = opool.tile([128, HSTG], BF, tag="osb", name="o_sb")
                    for hc in range(NCH // 2):
                        chk = half * (NCH // 2) + hc
                        p_ps = ps_mm.tile([128, FREE], FP, tag="mm512", name="p_ps")
                        csl = slice(chk * FREE, (chk + 1) * FREE)
                        for c2 in range(CT):
                            nc.tensor.matmul(
                                p_ps[:],
                                wps_sb[:, c2, o * 128:(o + 1) * 128],
                                a_sb[:, c2, csl],
                                start=(c2 == 0), stop=(c2 == CT - 1),
                            )
                        osl = slice(hc * FREE, (hc + 1) * FREE)
                        # Alternate evictions DVE/ScalarE to split load.
                        if has_p_bias:
                            if chk % 2 == 0:
                                nc.vector.tensor_scalar_add(
                                    o_sb[:, osl], in0=p_ps[:],
                                    scalar1=bp_sb[:, o:o + 1],
                                )
                            else:
                                nc.scalar.add(
                                    o_sb[:, osl], p_ps[:], add=bp_sb[:, o:o + 1]
                                )
                        else:
                            if chk % 2 == 0:
                                nc.vector.tensor_copy(o_sb[:, osl], p_ps[:])
                            else:
                                nc.scalar.copy(o_sb[:, osl], p_ps[:])
                        drain_interleave(1)
                    ysl = y_d[b, o * 128:(o + 1) * 128,
                              half * HSTG:(half + 1) * HSTG]
                    yq = nc.sync if b == BPC - 1 else nc.gpsimd
                    if b == BPC - 1 and o == CT - 1:
                        # Final tile: store per chunk so the last DMA is
                        # small — shortens the kernel tail.
                        for qs in range(HSTG // FREE):
                            yq.dma_start(
                                out=ysl[:, qs * FREE:(qs + 1) * FREE],
                                in_=o_sb[:, qs * FREE:(qs + 1) * FREE],
                            )
                    else:
                        yq.dma_start(out=ysl, in_=o_sb[:])
            # anything left (b==BPC-2 with fewer slots than ops)
            drain_interleave(len(interleave))

    nc.compile()
    return nc


_NC_CACHE = {}


def kernel(x, Wqkv, bqkv, Wp, bp):
    global LAST_RESULTS
    x = np.ascontiguousarray(np.asarray(x, dtype=np.float32))
    Wqkv = np.asarray(Wqkv, dtype=np.float32)
    bqkv = np.asarray(bqkv, dtype=np.float32)
    Wp = np.asarray(Wp, dtype=np.float32)
    bp = np.asarray(bp, dtype=np.float32)

    # Host-side weight layout prep (tiny, one-time).
    bf16 = ml_dtypes.bfloat16
    r = Wqkv.reshape(H, 1 + 2 * HD, C)
    wqT = np.ascontiguousarray(r[:, 0, :].T).astype(bf16)              # [C, H]
    wvT = np.ascontiguousarray(r[:, 1 + HD:, :].reshape(C, C).T).astype(bf16)
    wkO = np.ascontiguousarray(r[:, 1:1 + HD, :].reshape(C, C)).astype(bf16)
    wpT = np.ascontiguousarray(Wp.T).astype(bf16)                      # [C, o]
    rb = bqkv.reshape(H, 1 + 2 * HD)
    bq = np.ascontiguousarray(rb[:, 0].reshape(1, H)).astype(bf16)
    bkcol = np.ascontiguousarray(rb[:, 1:1 + HD].reshape(C)).astype(np.float32)
    bv = np.ascontiguousarray(rb[:, 1 + HD:].reshape(1, C)).astype(bf16)
    ch = np.arange(C)
    p128 = np.arange(128)
    eselB = ((p128[:, None] % 32) == (ch[None, :] // HD)).astype(bf16)
    eselT = ((np.arange(H)[:, None]) == (p128[None, :] % 32)).astype(np.float32)

    has_qkv_bias = bool(np.any(bqkv != 0.0))
    has_p_bias = bool(np.any(bp != 0.0))

    key = (has_qkv_bias, has_p_bias)
    if key not in _NC_CACHE:
        _NC_CACHE[key] = _build(*key)
    nc = _NC_CACHE[key]

    shared = {
        "wqT": wqT, "wvT": wvT, "wkO": wkO, "wpT": wpT,
        "eselB": eselB, "eselT": eselT,
        "bq": bq, "bkcol": bkcol, "bv": bv, "bp": bp,
    }
    # x ships to the device as bf16 (identical precision to the old
    # on-device cast; half the HBM traffic, no cast pipeline).
    xbf = np.ascontiguousarray(x.astype(bf16))
    in_maps = [
        {"x": xbf[i * BPC:(i + 1) * BPC], **shared} for i in range(NCORES)
    ]
    LAST_RESULTS = run_bass_kernel_spmd(nc, in_maps, list(range(NCORES)))
    out = np.concatenate(
        [LAST_RESULTS.results[i]["y"] for i in range(NCORES)], axis=0
    )
    return out.astype(np.float32)


if __name__ == "__main__":
    rng = np.random.default_rng(0)
    x = rng.standard_normal((B, C, N), dtype=np.float32)
    Wqkv = (rng.standard_normal((H * (1 + 2 * HD), C), dtype=np.float32) * 0.02)
    bqkv = np.zeros((H * (1 + 2 * HD),), np.float32)
    Wp = rng.standard_normal((C, C), dtype=np.float32) * 0.02
    bp = np.zeros((C,), np.float32)
    y = kernel(x, Wqkv, bqkv, Wp, bp)
    print("out", y.shape, y.dtype)


# revision 42
# speedup vs baseline: 1.0289x; 1.0289x over previous
"""BroadcastAttention Trainium2 kernel (8 NeuronCores, data-parallel over batch).

Math per sample (C=512, N=4096, H=8 heads, HD=64):
    qkv = Wqkv @ x            # [H*(1+2HD), N]
    q[h,n], k[h,d,n], v[h,d,n] split per head
    s = softmax(q over n)     # [H, N]
    ctx[h,d] = sum_n k[h,d,n]*s[h,n]
    out = Wp @ (relu(v)*ctx) + bp

Key algebraic restructuring vs the straightforward formulation: the dense
K projection (a full [512,512]@[512,4096] matmul per sample, one third of
the FLOPs) is never computed. Since ctx_h = Wk_h @ (x @ s_h), we compute
t[c,h] = sum_n x[c,n]*exp(q[h,n]) and apply Wk to the tiny [C,H] result
(eselB broadcast matmul + fused multiply-reduce against Wk), with 1/Z
applied via a small PE/DVE chain. ctx is folded into the P-phase weights
(wps = wp * ctx per contraction channel) so the V phase never waits on it.

Scheduling (derived from perfetto traces; ~175us HW, PE busy ~133us):
    - Matmul operands are bf16 (fp32 streams the PE at half rate); PSUM
      stays fp32. Steady 512-col matmuls run at the ~216ns streaming
      floor. x is converted to bf16 ON THE HOST (identical precision to
      the old on-device cast, half the HBM bytes, no stage/cast
      pipeline); y is stored bf16 and upcast on the host. Total rel err
      ~5e-3 vs the 2e-2 gate.
    - x loads DMA straight into x_sb slices on the Sync HWDGE queue in
      strict need-order (heads column-major for compute start, tails
      row-major so whole rows finish early). One queue only: concurrent
      loads on two queues split HBM bandwidth and delay first-needed
      tiles; GpSimd SWDGE transfers run at ~half the HWDGE rate (fine
      for weights, not for x).
    - x -> xT via XBAR dma_start_transpose, FULL-ROW only (each call
      has a ~5us fixed cost regardless of size), issued ONLY from the
      Sync queue: issuing from the Activation HWDGE queue produces
      corrupt data (measured ~50% error on affected samples), and
      fine-grained sliced transposes raced their readers.
    - The 8 t-accumulation groups (M=8 matmuls at distinct 32-col
      tile_position groups into one pre-zeroed PSUM bank) must run
      CONTIGUOUSLY on the PE: interleaving other matmul chains between
      the open accumulation subgroups corrupts ctx.
    - The Tile scheduler is dataflow-driven and hoists ready work into
      PE gaps: the next sample's loads are issued column-major (first
      column-block on GpSimd) so its q pass becomes runnable while the
      current sample's ctx chain drains; its transposes are drained
      into the P-phase emission. V7 out-tiles are emitted around the
      z/ctx chain's tiny PE ops, V7 evictions all go to Scalar so
      Vector runs the chain unobstructed, and the wps fold is pipelined
      per-column across Scalar/Vector (it is the last gate before P).
    - dma_start issue costs ~0.6us of sequencer time and the DMA ring
      paces issues at the transfer rate, so everything queued behind a
      blocked transpose ucode is delayed - queue assignment and
      emission order are the main scheduling tools. y stores issue from
      GpSimd so o_sb recycling never waits on transpose ucode.
    - After a device wedge (NRT errors), the next run can silently
      return wrong results on some cores - rerun before trusting a
      rel-err measurement.
"""

import sys

for _p in ("/opt/trn_rl_repo",):
    if _p not in sys.path:
        sys.path.insert(0, _p)

from contextlib import ExitStack

import ml_dtypes
import numpy as np

import concourse.bass as bass
import concourse.mybir as mybir
import concourse.tile as tile
from concourse import bacc
from concourse.bass_utils import run_bass_kernel_spmd

# Problem constants (hardcoded per contract; kernel.py must be self-contained).
B, C, N = 16, 512, 4096
H, HD = 8, 64
NCORES = 8
BPC = B // NCORES  # samples per core
CT = C // 128      # 4 contraction/partition tiles of 128
NT = N // 128      # 32 n-tiles
FREE = 512         # matmul moving free-dim chunk
NCH = N // FREE    # 8 chunks
FP = mybir.dt.float32
BF = mybir.dt.bfloat16

# Results of the last run (for test harness introspection).
LAST_RESULTS = None


def _build(has_qkv_bias: bool, has_p_bias: bool) -> bass.Bass:
    nc = bacc.Bacc("TRN2", target_bir_lowering=False, debug=False)

    x_d = nc.declare_dram_parameter("x", [BPC, C, N], BF, isOutput=False)
    wq_d = nc.declare_dram_parameter("wqT", [C, H], BF, isOutput=False)
    wv_d = nc.declare_dram_parameter("wvT", [C, C], BF, isOutput=False)
    wk_d = nc.declare_dram_parameter("wkO", [C, C], BF, isOutput=False)
    wp_d = nc.declare_dram_parameter("wpT", [C, C], BF, isOutput=False)
    eselB_d = nc.declare_dram_parameter("eselB", [128, C], BF, isOutput=False)
    eselT_d = nc.declare_dram_parameter("eselT", [H, 128], FP, isOutput=False)
    bq_d = nc.declare_dram_parameter("bq", [1, H], BF, isOutput=False)
    bkc_d = nc.declare_dram_parameter("bkcol", [C], FP, isOutput=False)
    bv_d = nc.declare_dram_parameter("bv", [1, C], BF, isOutput=False)
    bp_d = nc.declare_dram_parameter("bp", [C], FP, isOutput=False)
    y_d = nc.declare_dram_parameter("y", [BPC, C, N], BF, isOutput=True)

    AF = mybir.ActivationFunctionType
    OP = mybir.AluOpType

    with tile.TileContext(nc) as tc, ExitStack() as ctx:
        consts = ctx.enter_context(tc.tile_pool(name="consts", bufs=1))
        xpool = ctx.enter_context(tc.tile_pool(name="xpool", bufs=2))
        xtpool = ctx.enter_context(tc.tile_pool(name="xtpool", bufs=1))
        apool = ctx.enter_context(tc.tile_pool(name="apool", bufs=1))
        spool = ctx.enter_context(tc.tile_pool(name="spool", bufs=2))
        wpspool = ctx.enter_context(tc.tile_pool(name="wpspool", bufs=2))
        opool = ctx.enter_context(tc.tile_pool(name="opool", bufs=10))
        small = ctx.enter_context(tc.tile_pool(name="small", bufs=2))
        # 5 rotating matmul banks (the eselB finalize matmuls share the
        # same pool): deeper PSUM rotation hides the V/P chain-head
        # stationary loads + bank-acquire waits.
        ps_q = ctx.enter_context(tc.tile_pool(name="ps_q", bufs=2, space="PSUM"))
        ps_ctx = ctx.enter_context(tc.tile_pool(name="ps_ctx", bufs=1, space="PSUM"))
        ps_mm = ctx.enter_context(tc.tile_pool(name="ps_mm", bufs=5, space="PSUM"))

        # ---- constants / weights into SBUF ----
        # GpSimd (SWDGE) queue: startup weights, away from x on Sync.
        wq_sb = consts.tile([128, CT, H], BF)
        wv_sb = consts.tile([128, CT, C], BF)
        wk_sb = consts.tile([128, CT, C], BF)
        wp_sb = consts.tile([128, CT, C], BF)
        eselB_sb = consts.tile([128, C], BF)
        eselT_sb = consts.tile([H, 128], FP)
        ones_col = consts.tile([128, 1], FP)

        # All weights load upfront on GpSimd (SWDGE) — the slower SWDGE
        # transfer rate (~half of HWDGE) is fine for weights, and this
        # keeps the Sync HWDGE queue free for x loads + transposes.
        nc.gpsimd.memset(ones_col[:], 1.0)
        for wct in range(CT):
            wsl = slice(wct * 128, (wct + 1) * 128)
            nc.gpsimd.dma_start(out=wq_sb[:, wct, :], in_=wq_d[wsl, :])
            nc.gpsimd.dma_start(out=wv_sb[:, wct, :], in_=wv_d[wsl, :])
        nc.gpsimd.dma_start(out=eselB_sb[:], in_=eselB_d[:, :])
        nc.gpsimd.dma_start(out=eselT_sb[:], in_=eselT_d[:, :])
        for wct in range(CT):
            wsl = slice(wct * 128, (wct + 1) * 128)
            nc.gpsimd.dma_start(out=wk_sb[:, wct, :], in_=wk_d[wsl, :])
            nc.gpsimd.dma_start(out=wp_sb[:, wct, :], in_=wp_d[wsl, :])
        if has_qkv_bias:
            bq_sb = consts.tile([1, H], BF)
            bkc_sb = consts.tile([128, CT], FP)
            bv_sb = consts.tile([1, C], BF)
            ones_row = consts.tile([1, FREE], BF)
            nc.gpsimd.dma_start(out=bq_sb[:], in_=bq_d[:, :])
            nc.gpsimd.dma_start(
                out=bkc_sb[:], in_=bkc_d.rearrange("(o p) -> p o", p=128)
            )
            nc.gpsimd.dma_start(out=bv_sb[:], in_=bv_d[:, :])
            nc.gpsimd.memset(ones_row[:], 1.0)
        if has_p_bias:
            bp_sb = consts.tile([128, CT], FP)
            nc.gpsimd.dma_start(
                out=bp_sb[:], in_=bp_d.rearrange("(o p) -> p o", p=128)
            )

        # x arrives bf16 from the host (identical precision to the old
        # on-device fp32->bf16 cast, but half the HBM bytes and no
        # stage/cast pipeline at all). Loads go straight into x_sb
        # slices in [128, 1024] pieces, need-ordered. Each
        # dma_start_transpose has a ~5us fixed cost regardless of size,
        # so transposes stay FULL-ROW (4 per sample, Sync ONLY — the
        # Activation HWDGE queue corrupts transposes).
        QW = 1024  # quarter width

        def emit_ld(b, x_sb, r, p, queue):
            queue.dma_start(
                out=x_sb[:, r, p * QW:(p + 1) * QW],
                in_=x_d[b, r * 128:(r + 1) * 128, p * QW:(p + 1) * QW],
            )

        def emit_T(x_sb, xT_sb, r, queue=None):
            # XBAR transpose of one full x row-block (identical op to the
            # proven full-sample design, just issued earlier).
            (queue or nc.sync).dma_start_transpose(
                out=xT_sb[:, :, r * 128:(r + 1) * 128],
                in_=x_sb[:, r, :],
            )

        def emit_t_group(chk, scoresT, xT_sb, ctx_big):
            # 4 M=8 matmuls at distinct 32-col tile_position groups run
            # concurrently; accumulate into the pre-zeroed ctx_big bank.
            for j in range(4):
                nt = chk * 4 + j
                nc.tensor.matmul(
                    ctx_big[32 * j:32 * j + H, :],
                    scoresT[:, nt, :], xT_sb[:, nt, :],
                    start=False, stop=(chk == NCH - 1),
                    skip_group_check=True,
                    tile_position=(0, 32 * j),
                )

        def alloc_sample():
            xT_sb = xtpool.tile([128, NT, C], BF, tag="xT", name="xT_sb")
            scoresT = spool.tile([128, NT, H], BF, tag="scoresT", name="scoresT")
            a_sb = apool.tile([128, CT, N], BF, tag="a_sb", name="a_sb")
            wps_sb = wpspool.tile([128, CT, C], BF, tag="wps", name="wps_sb")
            ctx_big = ps_ctx.tile([128, C], FP, tag="ctx", name="ctx_big")
            nc.vector.memset(ctx_big[:], 0.0)
            return xT_sb, scoresT, a_sb, wps_sb, ctx_big

        x_sb = xpool.tile([128, CT, N], BF, tag="x_sb", name="x_sb")
        cur = alloc_sample()

        for b in range(BPC):
            xT_sb, scoresT, a_sb, wps_sb, ctx_big = cur

            if b == 0:
                # ALL of batch 0's x loads issue upfront on Sync (HWDGE
                # — SWDGE transfers at only ~half the rate) in strict
                # need-order: heads (cols 0-2048) column-major so the
                # chunk loop starts ASAP, then tails row-major so whole
                # rows complete early. The 4 full-row transposes follow
                # on Sync — each blocks the queue head until its row's
                # loads land, which is harmless (nothing else needs
                # Sync until the next sample's loads). Transposes MUST
                # issue from Sync: dma_start_transpose from the
                # Activation queue produces corrupt data (measured:
                # every sample whose odd rows transposed via Scalar
                # came out ~50% wrong; all-Sync samples were exact).
                for p in (0, 1):
                    for r in range(CT):
                        emit_ld(b, x_sb, r, p, nc.sync)
                for r in range(CT):
                    # one [128,2048] DMA per row tail: 12 total issues
                    # keeps the whole load burst inside the DMA ring's
                    # ~8-9 immediate-issue depth, so the Sync queue head
                    # reaches the transposes ~7us sooner.
                    nc.sync.dma_start(
                        out=x_sb[:, r, 2 * QW:4 * QW],
                        in_=x_d[b, r * 128:(r + 1) * 128, 2 * QW:4 * QW],
                    )
                for r in range(CT):
                    emit_T(x_sb, xT_sb, r)

            for chk in range(NCH):
                # ---- q pass for the 4 n-tiles of this chunk ----
                for j4 in range(4):
                    nt = chk * 4 + j4
                    nsl = slice(nt * 128, (nt + 1) * 128)
                    q_ps = ps_q.tile([128, H], FP, tag="q8", name="q_ps")
                    for ct in range(CT):
                        xsl = x_sb[:, ct, nsl]
                        last = (ct == CT - 1) and not has_qkv_bias
                        nc.tensor.matmul(
                            q_ps[:], xsl, wq_sb[:, ct, :],
                            start=(ct == 0), stop=last,
                        )
                    if has_qkv_bias:
                        nc.tensor.matmul(
                            q_ps[:], ones_row[:, 0:128], bq_sb[:],
                            start=False, stop=True,
                        )
                    nc.scalar.activation(
                        out=scoresT[:, nt, :], in_=q_ps[:], func=AF.Exp
                    )

                def emit_V(i_list, scalar_evict=False):
                    csl = slice(chk * FREE, (chk + 1) * FREE)
                    for i in i_list:
                        v_ps = ps_mm.tile(
                            [128, FREE], FP, tag="mm512", name="v_ps"
                        )
                        for ct in range(CT):
                            last = (ct == CT - 1) and not has_qkv_bias
                            nc.tensor.matmul(
                                v_ps[:],
                                wv_sb[:, ct, i * 128:(i + 1) * 128],
                                x_sb[:, ct, csl],
                                start=(ct == 0), stop=last,
                            )
                        if has_qkv_bias:
                            nc.tensor.matmul(
                                v_ps[:], bv_sb[:, i * 128:(i + 1) * 128],
                                ones_row[:], start=False, stop=True,
                            )
                        if i % 2 == 0 and not scalar_evict:
                            nc.vector.tensor_scalar_max(
                                out=a_sb[:, i, csl], in0=v_ps[:], scalar1=0.0
                            )
                        else:
                            nc.scalar.activation(
                                out=a_sb[:, i, csl], in_=v_ps[:], func=AF.Relu
                            )

                if chk < NCH - 1:
                    # ---- V phase for this chunk (no ctx dependency) ----
                    emit_V(range(CT))
                    continue

                # ---- last chunk: V out-tiles 0-2 run FIRST so the PE
                # chews them while the last transpose lands (t-groups
                # need all of xT); then the 8 t-groups contiguously
                # (other matmul chains interleaved between the open
                # tile_position accumulation groups corrupt ctx); then
                # the z/ctx chain with V(3) slotted between its tiny PE
                # ops to cover the DVE round trips. V7's evictions all
                # go to Scalar so Vector runs the chain unobstructed. ----
                zpart = small.tile([128, H], FP, tag="zpart", name="zpart")
                nc.vector.reduce_sum(
                    out=zpart[:],
                    in_=scoresT[:].rearrange("p nt h -> p h nt"),
                    axis=mybir.AxisListType.X,
                )
                emit_V([0, 1, 2], scalar_evict=True)
                for g in range(NCH):
                    emit_t_group(g, scoresT, xT_sb, ctx_big)
                z_ps = ps_q.tile([H, 1], FP, tag="q8", name="z_ps")
                nc.tensor.matmul(
                    z_ps[:], zpart[:], ones_col[:], start=True, stop=True
                )
                invz = small.tile([H, 1], FP, tag="invz", name="invz")
                nc.vector.reciprocal(out=invz[:], in_=z_ps[:])
                emit_V([3], scalar_evict=True)
                zrow_ps = ps_q.tile([128, 1], FP, tag="q8", name="zrow_ps")
                nc.tensor.matmul(
                    zrow_ps[:], eselT_sb[:], invz[:], start=True, stop=True
                )
                zrow = small.tile([128, 1], FP, tag="zrow", name="zrow")
                nc.vector.tensor_copy(out=zrow[:], in_=zrow_ps[:])
                ctxcopy = small.tile([128, C], BF, tag="ctxcopy", name="ctxcopy")
                nc.vector.tensor_scalar_mul(
                    out=ctxcopy[:], in0=ctx_big[:], scalar1=zrow[:]
                )
                # ctx[ch] = sum_c Wk[ch,c] * t[c,h(ch)]: broadcast t to
                # all channels of its head (eselB matmul, which also
                # combines the 4 substreams), then a fused elementwise
                # multiply-reduce against Wk.
                # ctxv held as 4 separate [128,1] tiles so the wps fold
                # can scale with full-tile scalar operands on either
                # engine; the fold is pipelined right behind each
                # reduce (it is the last gate before the P phase).
                for i in range(CT):
                    tb_ps = ps_mm.tile([128, C], FP, tag="mm512", name="tb_ps")
                    nc.tensor.matmul(
                        tb_ps[:], eselB_sb[:, i * 128:(i + 1) * 128],
                        ctxcopy[:], start=True, stop=True,
                    )
                    junk = small.tile([128, C], BF, tag="junk", name="junk")
                    nc.vector.tensor_tensor(
                        out=junk[:], in0=tb_ps[:], in1=wk_sb[:, i, :],
                        op=OP.mult,
                    )
                    ctxv_i = small.tile([128, 1], FP, tag="ctxv", name="ctxv")
                    nc.vector.reduce_sum(
                        out=ctxv_i[:], in_=junk[:],
                        axis=mybir.AxisListType.X,
                    )
                    if has_qkv_bias:
                        ctxvb_i = small.tile(
                            [128, 1], FP, tag="ctxv", name="ctxvb"
                        )
                        nc.vector.tensor_tensor(
                            out=ctxvb_i[:], in0=ctxv_i[:],
                            in1=bkc_sb[:, i:i + 1], op=OP.add,
                        )
                        ctxv_i = ctxvb_i
                    if i % 2 == 0:
                        nc.scalar.activation(
                            out=wps_sb[:, i, :], in_=wp_sb[:, i, :],
                            func=AF.Identity, scale=ctxv_i[:],
                        )
                    else:
                        nc.vector.tensor_scalar_mul(
                            out=wps_sb[:, i, :], in0=wp_sb[:, i, :],
                            scalar1=ctxv_i[:],
                        )

            # ---- prefetch next sample: HBM loads row-major on the idle
            # GpSimd SWDGE queue; casts + full-row transposes are
            # interleaved into the P-phase chunk slots below (32 slots
            # hold 16 casts + 4 transposes), so the next chunk loop
            # starts with x and xT fully resident. ----
            nxt = b + 1 if b + 1 < BPC else None
            interleave = []
            if nxt is not None:
                # Column-major, first column-block on GpSimd: the next
                # sample's q pass for early chunks becomes runnable
                # while this sample's ctx chain drains, and the
                # dataflow scheduler hoists it into those PE gaps.
                x_sb = xpool.tile([128, CT, N], BF, tag="x_sb", name="x_sb")
                cur = alloc_sample()
                nxT = cur[0]
                for p in range(4):
                    for r in range(CT):
                        emit_ld(nxt, x_sb, r, p,
                                nc.gpsimd if p == 0 else nc.sync)
                for r in range(CT):
                    interleave.append((emit_T, (x_sb, nxT, r)))
            ivi = [0]

            def drain_interleave(k=1):
                for _ in range(k):
                    if ivi[0] < len(interleave):
                        fn, args = interleave[ivi[0]]
                        fn(*args)
                        ivi[0] += 1

            # ---- P phase: output projection (ctx-folded weights) ----
            HSTG = N // 2
            for o in range(CT):
                for half in range(2):
                    o_sb = opool.tile([128, HSTG], BF, tag="osb", name="o_sb")
                    for hc in range(NCH // 2):
                        chk = half * (NCH // 2) + hc
                        p_ps = ps_mm.tile([128, FREE], FP, tag="mm512", name="p_ps")
                        csl = slice(chk * FREE, (chk + 1) * FREE)
                        for c2 in range(CT):
                            nc.tensor.matmul(
                                p_ps[:],
                                wps_sb[:, c2, o * 128:(o + 1) * 128],
                                a_sb[:, c2, csl],
                                start=(c2 == 0), stop=(c2 == CT - 1),
                            )
                        osl = slice(hc * FREE, (hc + 1) * FREE)
                        # Alternate evictions DVE/ScalarE to split load.
                        if has_p_bias:
                            if chk % 2 == 0:
                                nc.vector.tensor_scalar_add(
                                    o_sb[:, osl], in0=p_ps[:],
                                    scalar1=bp_sb[:, o:o + 1],
                                )
                            else:
                                nc.scalar.add(
                                    o_sb[:, osl], p_ps[:], add=bp_sb[:, o:o + 1]
                                )
                        else:
                            if chk % 2 == 0:
                                nc.vector.tensor_copy(o_sb[:, osl], p_ps[:])
                            else:
                                nc.scalar.copy(o_sb[:, osl], p_ps[:])
                        drain_interleave(1)
                    ysl = y_d[b, o * 128:(o + 1) * 128,
                              half * HSTG:(half + 1) * HSTG]
                    yq = nc.sync if b == BPC - 1 else nc.gpsimd
                    if b == BPC - 1 and o == CT - 1:
                        # Final tile: store per chunk so the last DMA is
                        # small — shortens the kernel tail.
                        for qs in range(HSTG // FREE):
                            yq.dma_start(
                                out=ysl[:, qs * FREE:(qs + 1) * FREE],
                                in_=o_sb[:, qs * FREE:(qs + 1) * FREE],
                            )
                    else:
                        yq.dma_start(out=ysl, in_=o_sb[:])
            # anything left (b==BPC-2 with fewer slots than ops)
            drain_interleave(len(interleave))

    nc.compile()
    return nc


_NC_CACHE = {}


def kernel(x, Wqkv, bqkv, Wp, bp):
    global LAST_RESULTS
    x = np.ascontiguousarray(np.asarray(x, dtype=np.float32))
    Wqkv = np.asarray(Wqkv, dtype=np.float32)
    bqkv = np.asarray(bqkv, dtype=np.float32)
    Wp = np.asarray(Wp, dtype=np.float32)
    bp = np.asarray(bp, dtype=np.float32)

    # Host-side weight layout prep (tiny, one-time).
    bf16 = ml_dtypes.bfloat16
    r = Wqkv.reshape(H, 1 + 2 * HD, C)
    wqT = np.ascontiguousarray(r[:, 0, :].T).astype(bf16)              # [C, H]
    wvT = np.ascontiguousarray(r[:, 1 + HD:, :].reshape(C, C).T).astype(bf16)
    wkO = np.ascontiguousarray(r[:, 1:1 + HD, :].reshape(C, C)).astype(bf16)
    wpT = np.ascontiguousarray(Wp.T).astype(bf16)                      # [C, o]
    rb = bqkv.reshape(H, 1 + 2 * HD)
    bq = np.ascontiguousarray(rb[:, 0].reshape(1, H)).astype(bf16)
    bkcol = np.ascontiguousarray(rb[:, 1:1 + HD].reshape(C)).astype(np.float32)
    bv = np.ascontiguousarray(rb[:, 1 + HD:].reshape(1, C)).astype(bf16)
    ch = np.arange(C)
    p128 = np.arange(128)
    eselB = ((p128[:, None] % 32) == (ch[None, :] // HD)).astype(bf16)
    eselT = ((np.arange(H)[:, None]) == (p128[None, :] % 32)).astype(np.float32)

    has_qkv_bias = bool(np.any(bqkv != 0.0))
    has_p_bias = bool(np.any(bp != 0.0))

    key = (has_qkv_bias, has_p_bias)
    if key not in _NC_CACHE:
        _NC_CACHE[key] = _build(*key)
    nc = _NC_CACHE[key]

    shared = {
        "wqT": wqT, "wvT": wvT, "wkO": wkO, "wpT": wpT,
        "eselB": eselB, "eselT": eselT,
        "bq": bq, "bkcol": bkcol, "bv": bv, "bp": bp,
    }
    # x ships to the device as bf16 (identical precision to the old
    # on-device cast; half the HBM traffic, no cast pipeline).
    xbf = np.ascontiguousarray(x.astype(bf16))
    in_maps = [
        {"x": xbf[i * BPC:(i + 1) * BPC], **shared} for i in range(NCORES)
    ]
    LAST_RESULTS = run_bass_kernel_spmd(nc, in_maps, list(range(NCORES)))
    out = np.concatenate(
        [LAST_RESULTS.results[i]["y"] for i in range(NCORES)], axis=0
    )
    return out.astype(np.float32)


if __name__ == "__main__":
    rng = np.random.default_rng(0)
    x = rng.standard_normal((B, C, N), dtype=np.float32)
    Wqkv = (rng.standard_normal((H * (1 + 2 * HD), C), dtype=np.float32) * 0.02)
    bqkv = np.zeros((H * (1 + 2 * HD),), np.float32)
    Wp = rng.standard_normal((C, C), dtype=np.float32) * 0.02
    bp = np.zeros((C,), np.float32)
    y = kernel(x, Wqkv, bqkv, Wp, bp)
    print("out", y.shape, y.dtype)
